# revision 7
# baseline (speedup 1.0000x reference)
"""Grouped-expert SwiGLU (MoE) Bass kernel for 8 TRN2 NeuronCores.

Problem: tokens pre-sorted by expert with per-expert counts; for expert e's
token slice xs: y = (silu(xs @ G_e^T) * (xs @ U_e^T)) @ D_e^T.

Strategy (all host logic; device program is uniform SPMD across 8 cores):
  * Tokens are split into 512-token blocks (counts are multiples of 512).
  * The 32 blocks are decomposed into 8 pieces of 3 blocks + 8 pieces of
    1 block, each piece single-expert; every core gets one 3-piece and one
    1-piece => exactly 2048 tokens/core, perfectly balanced compute.
  * All operands are cast to bf16 on host (PE rate identical to f32r, but
    half the HBM traffic / SBUF footprint; abs accuracy ~4e-3 rel, well
    inside the 2e-2 gate). PSUM accumulation stays f32.
  * Every DMA source is pre-packed on host into the exact [partition,
    free] contiguous layout the device needs.
  * Device schedule targets a gap-free PE stream at the bf16 roofline
    (1 row/cycle @ 2.4 GHz => ~327.7 us/core):
      - warmup: a short stream of dummy matmuls on a zeroed tile runs
        while the first x/weight DMAs are in flight, so the PE clock is
        fully ramped (p-state) when real data lands;
      - phase 1 of BOTH pieces is interleaved per hb tile, flattening the
        gate/up weight-stream demand to ~75 GB/s and removing the
        piece-boundary stall;
      - down-proj weights prefetch on a combined rolling window across
        both pieces' phase 2;
      - y is written per output chunk (the paired partition-strided write
        lowers to a slow single-engine descriptor chain - avoided), and
        the final chunk's drain is split into two half copies + writes to
        minimize the post-compute tail.
"""

import numpy as np
import ml_dtypes

import concourse.tile as tile
from concourse import bacc, mybir
from concourse.bass_utils import run_bass_kernel_spmd

BF16 = ml_dtypes.bfloat16
TB = 512  # token block
NCORES = 8
NWARM = 12  # warmup matmuls (512 rows each) to ramp the PE clock

_PROGRAM_CACHE = {}


# --------------------------------------------------------------------------
# device program
# --------------------------------------------------------------------------
def build_program(piece_sizes, dim, hid, reps=1):
    """Uniform per-core program: for each piece i of piece_sizes[i] blocks,
    compute SwiGLU of its tokens with weight set i.

    Inputs (all bf16, host-prepacked so each DMA is contiguous/partition):
      xb        [nblk, 128, KD*TB]   x block b: [p][kd][t] = x[b*TB+t, kd*128+p]
      g{i},u{i} [HB, 128, KD*128]    [hb][p][kd][h] = W[hb*128+h, kd*128+p]
      d{i}      [NCH, 128, HB*128]   [cb][p][hb][c] = D[cb*128+c, hb*128+p]
    Output:
      y         [NCH, 128, T]  bf16  [cb][p][t] = out[t, cb*128+p]
    """
    key = (tuple(piece_sizes), dim, hid, reps)
    if key in _PROGRAM_CACHE:
        return _PROGRAM_CACHE[key]

    f32 = mybir.dt.float32
    bf16 = mybir.dt.bfloat16
    KD = dim // 128   # k-tiles for gate/up contraction
    HB = hid // 128   # h-tiles
    NCH = dim // 128  # output dim chunks
    NP = len(piece_sizes)
    nblk = sum(piece_sizes)
    T = nblk * TB
    piece_blk0 = [sum(piece_sizes[:i]) for i in range(NP)]

    nc = bacc.Bacc("TRN2", target_bir_lowering=False, debug=False, num_devices=NCORES)
    xb = nc.dram_tensor("xb", [nblk, 128, KD * TB], bf16, kind="ExternalInput").ap()
    gs, us, ds = [], [], []
    for i in range(NP):
        gs.append(nc.dram_tensor(f"g{i}", [HB, 128, KD * 128], bf16, kind="ExternalInput").ap())
        us.append(nc.dram_tensor(f"u{i}", [HB, 128, KD * 128], bf16, kind="ExternalInput").ap())
        ds.append(nc.dram_tensor(f"d{i}", [NCH, 128, HB * 128], bf16, kind="ExternalInput").ap())
    y = nc.dram_tensor("y", [NCH, 128, T], bf16, kind="ExternalOutput").ap()

    # down-proj column tiles stream in pairs (one transfer per 2 chunks)
    PD = 2 if NCH % 2 == 0 else 1
    NPR = NCH // PD
    chunk_plan = [(pi, pr) for pi in range(NP) for pr in range(NPR)]
    PFP = min(4, len(chunk_plan))  # rolling dw prefetch depth

    with tile.TileContext(nc) as tc:
        with (
            tc.tile_pool(name="xp", bufs=nblk) as xp,
            tc.tile_pool(name="h1p", bufs=1) as h1p,
            tc.tile_pool(name="wp", bufs=4) as wp,
            tc.tile_pool(name="dwp", bufs=5) as dwp,
            tc.tile_pool(name="actp", bufs=3) as actp,
            tc.tile_pool(name="otp", bufs=3) as otp,
            tc.tile_pool(name="warmp", bufs=1) as warmp,
            # pool slots are per-tag: psgu holds psg+psu tags (2 bufs each =
            # 4 banks), psop 3 banks, warm psum 1 bank -> all 8 PSUM banks
            tc.tile_pool(name="psgu", bufs=2, space="PSUM") as psgu,
            tc.tile_pool(name="psop", bufs=3, space="PSUM") as psop,
            tc.tile_pool(name="wpsp", bufs=1, space="PSUM") as wpsp,
        ):
          for _rep in range(reps):
            # ---- PE warmup: dummy matmuls on a zeroed tile keep the PE
            # busy (and its clock ramping to the full p-state) while the
            # startup-critical x/weight DMAs are still in flight. Results
            # land in the spare 8th PSUM bank and are never read.
            warm = warmp.tile([128, TB], bf16, tag="warm")
            nc.gpsimd.memset(warm, 0.0)
            wps = wpsp.tile([128, TB], f32, tag="wps")
            for _ in range(NWARM):
                nc.tensor.matmul(wps, warm[:, :128], warm, start=True, stop=True)

            # ---- x loads, all issued up front. The startup is HBM-supply
            # bound (~8 cores pull x + weights concurrently), so block 0 is
            # quarter-split (small dependency units => the PE starts on the
            # first 512KB) and its quarters go to two different queue rings
            # for a larger share of the DMA fabric; later blocks are single
            # transfers queued strictly behind on the sync ring (a dma_start
            # costs ~700ns of sequencer issue time - fewer is better).
            xws = []
            for b in range(nblk):
                xw = xp.tile([128, KD, TB], bf16, tag="x")
                nsplit = 4 if b == 0 else 1
                q = KD // nsplit
                for j in range(nsplit):
                    eng = nc.gpsimd if (b == 0 and j % 2 == 1) else nc.sync
                    eng.dma_start(
                        out=xw[:, j * q : (j + 1) * q, :],
                        in_=xb[
                            b, :, j * q * TB : (j + 1) * q * TB
                        ].rearrange("p (kd t) -> p kd t", t=TB),
                    )
                xws.append(xw)

            h1s = []
            for pi, sz in enumerate(piece_sizes):
                h1 = h1p.tile([128, HB, sz * TB], bf16, tag=f"h1_{pi}")
                h1s.append(h1)

            dws = {}

            def dw_load(ci):
                pi, pr = chunk_plan[ci]
                dw = dwp.tile([128, PD, HB, 128], bf16, tag="dw")
                nc.scalar.dma_start(
                    out=dw,
                    in_=ds[pi][pr * PD : (pr + 1) * PD].rearrange(
                        "d p (hb c) -> p d hb c", c=128
                    ),
                )
                dws[ci] = dw

            # ---- phase 1: h1[h, t] = silu(G^T x) * (U^T x), both pieces
            # interleaved per hb so the gate/up weight stream is flat
            # (~1MB per piece per hb, amortized over all blocks).
            for hb in range(HB):
                for pi, sz in enumerate(piece_sizes):
                    gw = wp.tile([128, KD, 128], bf16, tag="gw")
                    uw = wp.tile([128, KD, 128], bf16, tag="uw")
                    # the startup-critical first weights are half-split so
                    # the first matmul group isn't gated on a full 512KB
                    nsp = 2 if (hb == 0 and pi == 0) else 1
                    ws = KD // nsp
                    for w, src in ((gw, gs[pi]), (uw, us[pi])):
                        for j in range(nsp):
                            nc.scalar.dma_start(
                                out=w[:, j * ws : (j + 1) * ws, :],
                                in_=src[
                                    hb, :, j * ws * 128 : (j + 1) * ws * 128
                                ].rearrange("p (kd h) -> p kd h", h=128),
                            )
                    for tb in range(sz):
                        xw = xws[piece_blk0[pi] + tb]
                        psg = psgu.tile([128, TB], f32, tag="psg")
                        psu = psgu.tile([128, TB], f32, tag="psu")
                        for kd in range(KD):
                            nc.tensor.matmul(
                                psg,
                                gw[:, kd, :],
                                xw[:, kd, :],
                                start=(kd == 0),
                                stop=(kd == KD - 1),
                            )
                        for kd in range(KD):
                            nc.tensor.matmul(
                                psu,
                                uw[:, kd, :],
                                xw[:, kd, :],
                                start=(kd == 0),
                                stop=(kd == KD - 1),
                            )
                        act = actp.tile([128, TB], f32, tag="act")
                        nc.scalar.activation(
                            act, psg, mybir.ActivationFunctionType.Silu
                        )
                        nc.vector.tensor_mul(
                            h1s[pi][:, hb, tb * TB : (tb + 1) * TB], act, psu
                        )
                # hoisted dw prefetch: spread over the last two hb
                # iterations so phase 2 starts with a deep resident
                # pipeline without starving the gate/up stream.
                if hb == HB - 2:
                    for ci in range(min(2, PFP)):
                        dw_load(ci)
                if hb == HB - 1:
                    for ci in range(min(2, PFP), PFP):
                        dw_load(ci)

            # ---- phase 2: yT[c, t] = sum_h D^T[h, c] * h1[h, t], pieces in
            # order (the small piece last => small exposed drain). dw pairs
            # roll PFP ahead on the scalar queue (idle during phase 2).
            for ci, (pi, pr) in enumerate(chunk_plan):
                if ci + PFP < len(chunk_plan):
                    dw_load(ci + PFP)
                dw = dws.pop(ci)
                sz = piece_sizes[pi]
                Tp = sz * TB
                t_lo = piece_blk0[pi] * TB
                h1 = h1s[pi]
                ntc = Tp // 512
                for s in range(PD):
                    dcb = pr * PD + s
                    tail = ci == len(chunk_plan) - 1 and s == PD - 1
                    otd = otp.tile([128, Tp], bf16, tag=f"ot{pi}")
                    for tcol in range(ntc):
                        lo = tcol * 512
                        if tail and tcol == ntc - 1:
                            # final drain: two 256-row psum groups so the
                            # cast + write of half 0 overlap half 1's
                            # matmuls; the two writes issue on different
                            # queues so their descriptor-generation
                            # (~600ns each) also overlaps
                            for hf, eng in ((0, nc.sync), (1, nc.gpsimd)):
                                pso = psop.tile([128, 512], f32, tag="pso")
                                c0 = lo + hf * 256
                                for hbx in range(HB):
                                    nc.tensor.matmul(
                                        pso[:, :256],
                                        dw[:, s, hbx, :],
                                        h1[:, hbx, c0 : c0 + 256],
                                        start=(hbx == 0),
                                        stop=(hbx == HB - 1),
                                    )
                                nc.vector.tensor_copy(
                                    otd[:, c0 : c0 + 256], pso[:, :256]
                                )
                                eng.dma_start(
                                    out=y[dcb, :, t_lo + c0 : t_lo + c0 + 256],
                                    in_=otd[:, c0 : c0 + 256],
                                )
                            continue
                        pso = psop.tile([128, 512], f32, tag="pso")
                        for hbx in range(HB):
                            nc.tensor.matmul(
                                pso,
                                dw[:, s, hbx, :],
                                h1[:, hbx, lo : lo + 512],
                                start=(hbx == 0),
                                stop=(hbx == HB - 1),
                            )
                        nc.vector.tensor_copy(otd[:, lo : lo + 512], pso)
                        if tail:
                            nc.sync.dma_start(
                                out=y[dcb, :, t_lo + lo : t_lo + lo + 512],
                                in_=otd[:, lo : lo + 512],
                            )
                    if not tail:
                        nc.sync.dma_start(out=y[dcb, :, t_lo : t_lo + Tp], in_=otd)
    nc.move_matmul_waits_to_ldweights()
    nc.compile()
    _PROGRAM_CACHE[key] = nc
    return nc


# --------------------------------------------------------------------------
# host-side planning
# --------------------------------------------------------------------------
def plan_pieces(block_counts):
    """Decompose per-expert block counts into 8 cores x uniform piece sizes.

    Returns (piece_sizes, plans) where plans[core] = [(expert, block_start,
    nblocks), ...] with block_start in global padded block coordinates.
    Tries the (3, 1) split (balanced, min weight traffic); falls back to
    single-block pieces.
    """
    E = len(block_counts)
    starts = np.zeros(E, dtype=np.int64)
    np.cumsum(block_counts[:-1], out=starts[1:])
    total = int(np.sum(block_counts))

    if total == 4 * NCORES:
        # try k3[e] three-pieces + k1[e] singles with sum(k3) == 8
        k3 = [int(c) // 3 for c in block_counts]
        while sum(k3) > NCORES:
            e = max(range(E), key=lambda i: k3[i])
            k3[e] -= 1
        if sum(k3) == NCORES:
            threes, ones = [], []
            for e in range(E):
                b = int(block_counts[e])
                s = int(starts[e])
                for _ in range(k3[e]):
                    threes.append((e, s, 3))
                    s += 3
                while s < int(starts[e]) + b:
                    ones.append((e, s, 1))
                    s += 1
            assert len(threes) == NCORES and len(ones) == NCORES
            # pair same-expert pieces on the same core where possible; big
            # piece first, small last (the final drain of phase 2 is then
            # a single 512-token chunk)
            plans = []
            used1 = [False] * NCORES
            for t in threes:
                j = next(
                    (
                        i
                        for i in range(NCORES)
                        if not used1[i] and ones[i][0] == t[0]
                    ),
                    None,
                )
                if j is None:
                    j = next(i for i in range(NCORES) if not used1[i])
                used1[j] = True
                plans.append([t, ones[j]])
            return (3, 1), plans

    # fallback: single-block pieces, padded to a multiple of NCORES with
    # dummy zero blocks (expert 0 weights, output discarded)
    per_core = -(-total // NCORES)
    pieces = []
    for e in range(E):
        for b in range(int(block_counts[e])):
            pieces.append((e, int(starts[e]) + b, 1))
    while len(pieces) < per_core * NCORES:
        pieces.append((0, -1, 1))  # dummy
    plans = [pieces[c * per_core : (c + 1) * per_core] for c in range(NCORES)]
    return tuple([1] * per_core), plans


def _pack_gu(w, KD, HB):
    """[hid, dim] f32 -> [HB, 128, KD*128] bf16 with
    out[hb, p, kd*128+h] = w[hb*128+h, kd*128+p]."""
    hid, dim = w.shape
    arr = w.reshape(HB, 128, KD, 128).transpose(0, 3, 2, 1)  # hb, p, kd, h
    return np.ascontiguousarray(arr.astype(BF16).reshape(HB, 128, KD * 128))


def _pack_d(w, NCH, HB):
    """[dim, hid] f32 -> [NCH, 128, HB*128] bf16 with
    out[cb, p, hb*128+c] = w[cb*128+c, hb*128+p]."""
    dim, hid = w.shape
    arr = w.reshape(NCH, 128, HB, 128).transpose(0, 3, 2, 1)  # cb, p, hb, c
    return np.ascontiguousarray(arr.astype(BF16).reshape(NCH, 128, HB * 128))


def _pack_x(xc, KD):
    """[T, dim] f32 -> [nblk, 128, KD*TB] bf16 with
    out[b, p, kd*TB+t] = x[b*TB+t, kd*128+p]."""
    T, dim = xc.shape
    nblk = T // TB
    arr = xc.reshape(nblk, TB, KD, 128).transpose(0, 3, 2, 1)  # b, p, kd, t
    return np.ascontiguousarray(arr.astype(BF16).reshape(nblk, 128, KD * TB))


def prepare(x, gate_proj, up_proj, down_proj, num_tokens_per_expert):
    """Host-side planning + per-core input construction.

    Returns (piece_sizes, plans, in_maps, scatter_info).
    """
    x = np.ascontiguousarray(np.asarray(x, dtype=np.float32))
    gate_proj = np.asarray(gate_proj, dtype=np.float32)
    up_proj = np.asarray(up_proj, dtype=np.float32)
    down_proj = np.asarray(down_proj, dtype=np.float32)
    counts = np.asarray(num_tokens_per_expert).astype(np.int64)

    T, dim = x.shape
    E, hid, _ = gate_proj.shape
    KD, HB, NCH = dim // 128, hid // 128, dim // 128

    # ---- pad each expert's token segment to a multiple of TB (no-op for the
    # staged problem where every count is already a multiple of 512)
    offs = np.concatenate([[0], np.cumsum(counts)])
    pad_counts = ((counts + TB - 1) // TB) * TB
    if np.array_equal(pad_counts, counts):
        x_pad = x
        pad_offs = offs
        padded = False
    else:
        pad_offs = np.concatenate([[0], np.cumsum(pad_counts)])
        x_pad = np.zeros((int(pad_offs[-1]), dim), dtype=np.float32)
        for e in range(E):
            x_pad[pad_offs[e] : pad_offs[e] + counts[e]] = x[offs[e] : offs[e + 1]]
        padded = True

    block_counts = pad_counts // TB
    piece_sizes, plans = plan_pieces(block_counts)

    # ---- per-expert packed bf16 weights (shared across cores)
    GP = [_pack_gu(gate_proj[e], KD, HB) for e in range(E)]
    UP = [_pack_gu(up_proj[e], KD, HB) for e in range(E)]
    DP = [_pack_d(down_proj[e], NCH, HB) for e in range(E)]

    in_maps = []
    for c in range(NCORES):
        plan = plans[c]
        xs = []
        for (e, bs, nb) in plan:
            if bs < 0:
                xs.append(np.zeros((nb * TB, dim), dtype=np.float32))
            else:
                xs.append(x_pad[bs * TB : (bs + nb) * TB])
        xc = np.concatenate(xs, axis=0) if len(xs) > 1 else xs[0]
        m = {"xb": _pack_x(xc, KD)}
        for i, (e, bs, nb) in enumerate(plan):
            m[f"g{i}"] = GP[e]
            m[f"u{i}"] = UP[e]
            m[f"d{i}"] = DP[e]
        in_maps.append(m)

    scatter_info = (T, dim, E, offs, pad_offs, counts, padded)
    return piece_sizes, plans, in_maps, scatter_info


def scatter(per_core_y, plans, scatter_info):
    """Assemble the full output from per-core y arrays."""
    T, dim, E, offs, pad_offs, counts, padded = scatter_info
    out_pad = np.empty((int(pad_offs[-1]), dim), dtype=np.float32)
    for c in range(NCORES):
        yc = np.asarray(per_core_y[c]).astype(np.float32)  # [NCH, 128, T_core]
        yc = yc.transpose(2, 0, 1).reshape(yc.shape[2], dim)  # [T_core, dim]
        t = 0
        for (e, bs, nb) in plans[c]:
            if bs >= 0:
                out_pad[bs * TB : (bs + nb) * TB] = yc[t : t + nb * TB]
            t += nb * TB
    if not padded:
        return out_pad
    out = np.empty((T, dim), dtype=np.float32)
    for e in range(E):
        out[offs[e] : offs[e + 1]] = out_pad[pad_offs[e] : pad_offs[e] + counts[e]]
    return out


def kernel(x, gate_proj, up_proj, down_proj, num_tokens_per_expert):
    piece_sizes, plans, in_maps, scatter_info = prepare(
        x, gate_proj, up_proj, down_proj, num_tokens_per_expert
    )
    dim = scatter_info[1]
    hid = np.asarray(gate_proj).shape[1]
    nc = build_program(piece_sizes, dim, hid)
    res = run_bass_kernel_spmd(nc, in_maps, core_ids=list(range(NCORES)))
    return scatter([res.results[c]["y"] for c in range(NCORES)], plans, scatter_info)


# revision 13
# speedup vs baseline: 1.0127x; 1.0127x over previous
"""Grouped-expert SwiGLU (MoE) Bass kernel for 8 TRN2 NeuronCores.

Problem: tokens pre-sorted by expert with per-expert counts; for expert e's
token slice xs: y = (silu(xs @ G_e^T) * (xs @ U_e^T)) @ D_e^T.

Strategy (all host logic; device program is uniform SPMD across 8 cores):
  * Tokens are split into 512-token blocks (counts are multiples of 512).
  * The 32 blocks are decomposed into 8 pieces of 3 blocks + 8 pieces of
    1 block, each piece single-expert; every core gets one 3-piece and one
    1-piece => exactly 2048 tokens/core, perfectly balanced compute.
  * All operands are cast to bf16 on host (PE rate identical to f32r, but
    half the HBM traffic / SBUF footprint; abs accuracy ~4e-3 rel, well
    inside the 2e-2 gate). PSUM accumulation stays f32.
  * Every DMA source is pre-packed on host into the exact [partition,
    free] contiguous layout the device needs.
  * Device schedule targets a gap-free PE stream at the bf16 roofline
    (1 row/cycle @ 2.4 GHz => ~327.7 us/core):
      - warmup: a short stream of dummy matmuls on a zeroed tile runs
        while the first x/weight DMAs are in flight, so the PE clock is
        fully ramped (p-state) when real data lands;
      - phase 1 of BOTH pieces is interleaved per hb tile, flattening the
        gate/up weight-stream demand to ~75 GB/s and removing the
        piece-boundary stall;
      - down-proj weights prefetch on a combined rolling window across
        both pieces' phase 2;
      - y is written per output chunk (the paired partition-strided write
        lowers to a slow single-engine descriptor chain - avoided), and
        the final chunk's drain is split into two half copies + writes to
        minimize the post-compute tail.
"""

import numpy as np
import ml_dtypes

import concourse.tile as tile
from concourse import bacc, mybir
from concourse.bass_utils import run_bass_kernel_spmd

BF16 = ml_dtypes.bfloat16
TB = 512  # token block
NCORES = 8
NWARM = 20  # warmup matmuls (512 rows each) to ramp the PE clock

_PROGRAM_CACHE = {}


# --------------------------------------------------------------------------
# device program
# --------------------------------------------------------------------------
def build_program(piece_sizes, dim, hid, reps=1):
    """Uniform per-core program: for each piece i of piece_sizes[i] blocks,
    compute SwiGLU of its tokens with weight set i.

    Inputs (all bf16, host-prepacked so each DMA is contiguous/partition):
      xb        [nblk, 128, KD*TB]   x block b: [p][kd][t] = x[b*TB+t, kd*128+p]
      g{i},u{i} [HB, 128, KD*128]    [hb][p][kd][h] = W[hb*128+h, kd*128+p]
      d{i}      [NCH, 128, HB*128]   [cb][p][hb][c] = D[cb*128+c, hb*128+p]
    Output:
      y         [NCH, 128, T]  bf16  [cb][p][t] = out[t, cb*128+p]
    """
    key = (tuple(piece_sizes), dim, hid, reps)
    if key in _PROGRAM_CACHE:
        return _PROGRAM_CACHE[key]

    f32 = mybir.dt.float32
    bf16 = mybir.dt.bfloat16
    KD = dim // 128   # k-tiles for gate/up contraction
    HB = hid // 128   # h-tiles
    NCH = dim // 128  # output dim chunks
    NP = len(piece_sizes)
    nblk = sum(piece_sizes)
    T = nblk * TB
    piece_blk0 = [sum(piece_sizes[:i]) for i in range(NP)]
    HP = 2 if HB % 2 == 0 else 1   # hb tiles per gate/up weight transfer
    NHP = HB // HP
    PD = 2 if NCH % 2 == 0 else 1  # output chunks per down-proj transfer
    NPR = NCH // PD

    # DMA efficiency note: the startup window is descriptor-throughput
    # bound (8 cores pull x + weights at once), and descriptor length =
    # per-partition contiguous run. The host packs weights hb-PAIR-major
    # (8KB runs) and down-proj pair-contiguous (4KB runs); x blocks load
    # unsplit (16KB runs).
    nc = bacc.Bacc("TRN2", target_bir_lowering=False, debug=False, num_devices=NCORES)
    xb = nc.dram_tensor("xb", [nblk, 128, KD * TB], bf16, kind="ExternalInput").ap()
    gs, us, ds = [], [], []
    for i in range(NP):
        gs.append(nc.dram_tensor(f"g{i}", [NHP, 128, HP * KD * 128], bf16, kind="ExternalInput").ap())
        us.append(nc.dram_tensor(f"u{i}", [NHP, 128, HP * KD * 128], bf16, kind="ExternalInput").ap())
        ds.append(nc.dram_tensor(f"d{i}", [NPR, 128, PD * HB * 128], bf16, kind="ExternalInput").ap())
    y = nc.dram_tensor("y", [NCH, 128, T], bf16, kind="ExternalOutput").ap()

    chunk_plan = [(pi, pr) for pi in range(NP) for pr in range(NPR)]
    PFP = min(4, len(chunk_plan))  # rolling dw prefetch depth

    with tile.TileContext(nc) as tc:
        with (
            tc.tile_pool(name="xp", bufs=nblk) as xp,
            tc.tile_pool(name="h1p", bufs=1) as h1p,
            tc.tile_pool(name="wp", bufs=3) as wp,
            tc.tile_pool(name="dwp", bufs=5) as dwp,
            tc.tile_pool(name="actp", bufs=3) as actp,
            tc.tile_pool(name="otp", bufs=3) as otp,
            tc.tile_pool(name="warmp", bufs=1) as warmp,
            # pool slots are per-tag: psgu holds psg+psu tags (2 bufs each =
            # 4 banks), psop 3 banks, warm psum 1 bank -> all 8 PSUM banks
            tc.tile_pool(name="psgu", bufs=2, space="PSUM") as psgu,
            tc.tile_pool(name="psop", bufs=3, space="PSUM") as psop,
            tc.tile_pool(name="wpsp", bufs=1, space="PSUM") as wpsp,
        ):
          for _rep in range(reps):
            # ---- PE warmup: dummy matmuls on an iota-filled tile keep the
            # PE busy while the startup-critical x/weight DMAs are in
            # flight. The data is nonzero/varied on purpose: the clock
            # boost (HAM) tracks array ACTIVITY, and all-zero matmuls were
            # observed to never leave the mid p-state. Results land in the
            # spare 8th PSUM bank and are never read.
            warm = warmp.tile([128, TB], bf16, tag="warm")
            nc.gpsimd.iota(
                warm,
                [[1, TB]],
                channel_multiplier=7,
                allow_small_or_imprecise_dtypes=True,
            )
            wps = wpsp.tile([128, TB], f32, tag="wps")
            for _ in range(NWARM):
                nc.tensor.matmul(wps, warm[:, :128], warm, start=True, stop=True)
            # a tiny dummy activation forces the compiler-inserted silu
            # table loads (and their table-data DMAs) to the very front;
            # otherwise the first real silu can stall on them and
            # backpressure the psg/psu psum rotation into a PE stall
            warma = warmp.tile([128, 8], f32, tag="warma")
            nc.vector.memset(warma, 0.0)
            nc.scalar.activation(warma, warma, mybir.ActivationFunctionType.Silu)

            # ---- x loads, all issued up front on the sync queue. Single
            # transfer per block: 16KB per-partition runs for descriptor
            # throughput; the warmup bridges the PE to the data.
            xws = []
            for b in range(nblk):
                xw = xp.tile([128, KD, TB], bf16, tag="x")
                nc.sync.dma_start(
                    out=xw,
                    in_=xb[b].rearrange("p (kd t) -> p kd t", t=TB),
                )
                xws.append(xw)

            h1s = []
            for pi, sz in enumerate(piece_sizes):
                h1 = h1p.tile([128, HB, sz * TB], bf16, tag=f"h1_{pi}")
                h1s.append(h1)

            dws = {}

            def dw_load(ci):
                pi, pr = chunk_plan[ci]
                dw = dwp.tile([128, PD, HB, 128], bf16, tag="dw")
                nc.scalar.dma_start(
                    out=dw,
                    in_=ds[pi][pr].rearrange("p (s hb c) -> p s hb c", s=PD, c=128),
                )
                dws[ci] = dw

            # ---- phase 1: h1[h, t] = silu(G^T x) * (U^T x), both pieces
            # interleaved per hb pair so the gate/up weight stream is flat
            # (~2MB per piece per pair, amortized over all blocks).
            for hp in range(NHP):
                for pi, sz in enumerate(piece_sizes):
                    gw = wp.tile([128, HP, KD, 128], bf16, tag="gw")
                    uw = wp.tile([128, HP, KD, 128], bf16, tag="uw")
                    nc.scalar.dma_start(
                        out=gw,
                        in_=gs[pi][hp].rearrange("p (i kd h) -> p i kd h", i=HP, h=128),
                    )
                    nc.scalar.dma_start(
                        out=uw,
                        in_=us[pi][hp].rearrange("p (i kd h) -> p i kd h", i=HP, h=128),
                    )
                    for i in range(HP):
                        hb = hp * HP + i
                        for tb in range(sz):
                            xw = xws[piece_blk0[pi] + tb]
                            psg = psgu.tile([128, TB], f32, tag="psg")
                            psu = psgu.tile([128, TB], f32, tag="psu")
                            for kd in range(KD):
                                nc.tensor.matmul(
                                    psg,
                                    gw[:, i, kd, :],
                                    xw[:, kd, :],
                                    start=(kd == 0),
                                    stop=(kd == KD - 1),
                                )
                            for kd in range(KD):
                                nc.tensor.matmul(
                                    psu,
                                    uw[:, i, kd, :],
                                    xw[:, kd, :],
                                    start=(kd == 0),
                                    stop=(kd == KD - 1),
                                )
                            act = actp.tile([128, TB], f32, tag="act")
                            nc.scalar.activation(
                                act, psg, mybir.ActivationFunctionType.Silu
                            )
                            nc.vector.tensor_mul(
                                h1s[pi][:, hb, tb * TB : (tb + 1) * TB], act, psu
                            )
                # hoisted dw prefetch: spread over the last two pair
                # iterations so phase 2 starts with a deep resident
                # pipeline without starving the gate/up stream.
                if hp == NHP - 2:
                    for ci in range(min(2, PFP)):
                        dw_load(ci)
                if hp == NHP - 1:
                    for ci in range(min(2, PFP), PFP):
                        dw_load(ci)

            # ---- phase 2: yT[c, t] = sum_h D^T[h, c] * h1[h, t], pieces in
            # order (the small piece last => small exposed drain). dw pairs
            # roll PFP ahead on the scalar queue (idle during phase 2).
            for ci, (pi, pr) in enumerate(chunk_plan):
                if ci + PFP < len(chunk_plan):
                    dw_load(ci + PFP)
                dw = dws.pop(ci)
                sz = piece_sizes[pi]
                Tp = sz * TB
                t_lo = piece_blk0[pi] * TB
                h1 = h1s[pi]
                ntc = Tp // 512
                for s in range(PD):
                    dcb = pr * PD + s
                    tail = ci == len(chunk_plan) - 1 and s == PD - 1
                    otd = otp.tile([128, Tp], bf16, tag=f"ot{pi}")
                    for tcol in range(ntc):
                        lo = tcol * 512
                        if tail and tcol == ntc - 1:
                            # final drain: two 256-row psum groups so the
                            # cast + write of half 0 overlap half 1's
                            # matmuls; the two writes issue on different
                            # queues so their descriptor-generation
                            # (~600ns each) also overlaps
                            for hf, eng in ((0, nc.sync), (1, nc.gpsimd)):
                                pso = psop.tile([128, 512], f32, tag="pso")
                                c0 = lo + hf * 256
                                for hbx in range(HB):
                                    nc.tensor.matmul(
                                        pso[:, :256],
                                        dw[:, s, hbx, :],
                                        h1[:, hbx, c0 : c0 + 256],
                                        start=(hbx == 0),
                                        stop=(hbx == HB - 1),
                                    )
                                nc.vector.tensor_copy(
                                    otd[:, c0 : c0 + 256], pso[:, :256]
                                )
                                eng.dma_start(
                                    out=y[dcb, :, t_lo + c0 : t_lo + c0 + 256],
                                    in_=otd[:, c0 : c0 + 256],
                                )
                            continue
                        pso = psop.tile([128, 512], f32, tag="pso")
                        for hbx in range(HB):
                            nc.tensor.matmul(
                                pso,
                                dw[:, s, hbx, :],
                                h1[:, hbx, lo : lo + 512],
                                start=(hbx == 0),
                                stop=(hbx == HB - 1),
                            )
                        nc.vector.tensor_copy(otd[:, lo : lo + 512], pso)
                        if tail:
                            nc.sync.dma_start(
                                out=y[dcb, :, t_lo + lo : t_lo + lo + 512],
                                in_=otd[:, lo : lo + 512],
                            )
                    if not tail:
                        nc.sync.dma_start(out=y[dcb, :, t_lo : t_lo + Tp], in_=otd)
    nc.move_matmul_waits_to_ldweights()
    nc.compile()
    _PROGRAM_CACHE[key] = nc
    return nc


# --------------------------------------------------------------------------
# host-side planning
# --------------------------------------------------------------------------
def plan_pieces(block_counts):
    """Decompose per-expert block counts into 8 cores x uniform piece sizes.

    Returns (piece_sizes, plans) where plans[core] = [(expert, block_start,
    nblocks), ...] with block_start in global padded block coordinates.
    Tries the (3, 1) split (balanced, min weight traffic); falls back to
    single-block pieces.
    """
    E = len(block_counts)
    starts = np.zeros(E, dtype=np.int64)
    np.cumsum(block_counts[:-1], out=starts[1:])
    total = int(np.sum(block_counts))

    if total == 4 * NCORES:
        # try k3[e] three-pieces + k1[e] singles with sum(k3) == 8
        k3 = [int(c) // 3 for c in block_counts]
        while sum(k3) > NCORES:
            e = max(range(E), key=lambda i: k3[i])
            k3[e] -= 1
        if sum(k3) == NCORES:
            threes, ones = [], []
            for e in range(E):
                b = int(block_counts[e])
                s = int(starts[e])
                for _ in range(k3[e]):
                    threes.append((e, s, 3))
                    s += 3
                while s < int(starts[e]) + b:
                    ones.append((e, s, 1))
                    s += 1
            assert len(threes) == NCORES and len(ones) == NCORES
            # pair same-expert pieces on the same core where possible; big
            # piece first, small last (the final drain of phase 2 is then
            # a single 512-token chunk)
            plans = []
            used1 = [False] * NCORES
            for t in threes:
                j = next(
                    (
                        i
                        for i in range(NCORES)
                        if not used1[i] and ones[i][0] == t[0]
                    ),
                    None,
                )
                if j is None:
                    j = next(i for i in range(NCORES) if not used1[i])
                used1[j] = True
                plans.append([t, ones[j]])
            return (3, 1), plans

    # fallback: single-block pieces, padded to a multiple of NCORES with
    # dummy zero blocks (expert 0 weights, output discarded)
    per_core = -(-total // NCORES)
    pieces = []
    for e in range(E):
        for b in range(int(block_counts[e])):
            pieces.append((e, int(starts[e]) + b, 1))
    while len(pieces) < per_core * NCORES:
        pieces.append((0, -1, 1))  # dummy
    plans = [pieces[c * per_core : (c + 1) * per_core] for c in range(NCORES)]
    return tuple([1] * per_core), plans


def _pack_gu(w, KD, HB):
    """[hid, dim] f32 -> [HB//HP, 128, HP*KD*128] bf16 (hb-PAIR-major, so a
    pair transfer is an 8KB contiguous run per partition) with
    out[hp, p, ((i*KD+kd)*128)+h] = w[(hp*HP+i)*128+h, kd*128+p]."""
    hid, dim = w.shape
    HP = 2 if HB % 2 == 0 else 1
    # (hp, i, h, kd, p128) -> (hp, p128, i, kd, h)
    arr = w.reshape(HB // HP, HP, 128, KD, 128).transpose(0, 4, 1, 3, 2)
    return np.ascontiguousarray(arr.astype(BF16).reshape(HB // HP, 128, HP * KD * 128))


def _pack_d(w, NCH, HB):
    """[dim, hid] f32 -> [NCH//PD, 128, PD*HB*128] bf16 (chunk-PAIR-major:
    4KB contiguous run per partition per transfer) with
    out[pr, p, ((s*HB+hb)*128)+c] = w[(pr*PD+s)*128+c, hb*128+p]."""
    dim, hid = w.shape
    PD = 2 if NCH % 2 == 0 else 1
    # (pr, s, c, hb, p128) -> (pr, p128, s, hb, c)
    arr = w.reshape(NCH // PD, PD, 128, HB, 128).transpose(0, 4, 1, 3, 2)
    return np.ascontiguousarray(arr.astype(BF16).reshape(NCH // PD, 128, PD * HB * 128))


def _pack_x(xc, KD):
    """[T, dim] f32 -> [nblk, 128, KD*TB] bf16 with
    out[b, p, kd*TB+t] = x[b*TB+t, kd*128+p]."""
    T, dim = xc.shape
    nblk = T // TB
    arr = xc.reshape(nblk, TB, KD, 128).transpose(0, 3, 2, 1)  # b, p, kd, t
    return np.ascontiguousarray(arr.astype(BF16).reshape(nblk, 128, KD * TB))


def prepare(x, gate_proj, up_proj, down_proj, num_tokens_per_expert):
    """Host-side planning + per-core input construction.

    Returns (piece_sizes, plans, in_maps, scatter_info).
    """
    x = np.ascontiguousarray(np.asarray(x, dtype=np.float32))
    gate_proj = np.asarray(gate_proj, dtype=np.float32)
    up_proj = np.asarray(up_proj, dtype=np.float32)
    down_proj = np.asarray(down_proj, dtype=np.float32)
    counts = np.asarray(num_tokens_per_expert).astype(np.int64)

    T, dim = x.shape
    E, hid, _ = gate_proj.shape
    KD, HB, NCH = dim // 128, hid // 128, dim // 128

    # ---- pad each expert's token segment to a multiple of TB (no-op for the
    # staged problem where every count is already a multiple of 512)
    offs = np.concatenate([[0], np.cumsum(counts)])
    pad_counts = ((counts + TB - 1) // TB) * TB
    if np.array_equal(pad_counts, counts):
        x_pad = x
        pad_offs = offs
        padded = False
    else:
        pad_offs = np.concatenate([[0], np.cumsum(pad_counts)])
        x_pad = np.zeros((int(pad_offs[-1]), dim), dtype=np.float32)
        for e in range(E):
            x_pad[pad_offs[e] : pad_offs[e] + counts[e]] = x[offs[e] : offs[e + 1]]
        padded = True

    block_counts = pad_counts // TB
    piece_sizes, plans = plan_pieces(block_counts)

    # ---- per-expert packed bf16 weights (shared across cores)
    GP = [_pack_gu(gate_proj[e], KD, HB) for e in range(E)]
    UP = [_pack_gu(up_proj[e], KD, HB) for e in range(E)]
    DP = [_pack_d(down_proj[e], NCH, HB) for e in range(E)]

    in_maps = []
    for c in range(NCORES):
        plan = plans[c]
        xs = []
        for (e, bs, nb) in plan:
            if bs < 0:
                xs.append(np.zeros((nb * TB, dim), dtype=np.float32))
            else:
                xs.append(x_pad[bs * TB : (bs + nb) * TB])
        xc = np.concatenate(xs, axis=0) if len(xs) > 1 else xs[0]
        m = {"xb": _pack_x(xc, KD)}
        for i, (e, bs, nb) in enumerate(plan):
            m[f"g{i}"] = GP[e]
            m[f"u{i}"] = UP[e]
            m[f"d{i}"] = DP[e]
        in_maps.append(m)

    scatter_info = (T, dim, E, offs, pad_offs, counts, padded)
    return piece_sizes, plans, in_maps, scatter_info


def scatter(per_core_y, plans, scatter_info):
    """Assemble the full output from per-core y arrays."""
    T, dim, E, offs, pad_offs, counts, padded = scatter_info
    out_pad = np.empty((int(pad_offs[-1]), dim), dtype=np.float32)
    for c in range(NCORES):
        yc = np.asarray(per_core_y[c]).astype(np.float32)  # [NCH, 128, T_core]
        yc = yc.transpose(2, 0, 1).reshape(yc.shape[2], dim)  # [T_core, dim]
        t = 0
        for (e, bs, nb) in plans[c]:
            if bs >= 0:
                out_pad[bs * TB : (bs + nb) * TB] = yc[t : t + nb * TB]
            t += nb * TB
    if not padded:
        return out_pad
    out = np.empty((T, dim), dtype=np.float32)
    for e in range(E):
        out[offs[e] : offs[e + 1]] = out_pad[pad_offs[e] : pad_offs[e] + counts[e]]
    return out


def kernel(x, gate_proj, up_proj, down_proj, num_tokens_per_expert):
    piece_sizes, plans, in_maps, scatter_info = prepare(
        x, gate_proj, up_proj, down_proj, num_tokens_per_expert
    )
    dim = scatter_info[1]
    hid = np.asarray(gate_proj).shape[1]
    nc = build_program(piece_sizes, dim, hid)
    res = run_bass_kernel_spmd(nc, in_maps, core_ids=list(range(NCORES)))
    return scatter([res.results[c]["y"] for c in range(NCORES)], plans, scatter_info)
